# revision 18
# baseline (speedup 1.0000x reference)
"""GQA attention (16 q-heads / 4 kv-heads, head_dim 64, T=2048, D=1024) on 8
Trainium2 NeuronCores.

Sharding: 8 shards = batch(2) x kv-group(4). Each core handles one batch
element and one whole GQA group (4 query heads + their shared kv head), and
computes a partial output projection; the host sums the 4 group-partials per
batch element.

v2 pipeline per core:
  Phase 1: B-tiles 15..0 (qkv proj, RMSNorm via DVE recip+Newton-rsqrt --
    keeps ACT on a single exp table set -- RoPE on gpsimd, PE transposes)
    merged with the S-pass of head-pair A: row-tiled concurrent score
    matmul pairs (heads at partitions 0-63 / 64-127 share the PE array via
    distinct row groups), pair-merged exp on ACT [128,2,N], with a share of
    k-tiles routed to a custom 2-op DVE exp (deg-4 poly of exp(x/32) then
    5 squarings, clamped).
  Phase 2: PV-pass of pair A (per-chunk psum accumulation, ones-row
    denominator, DRAM-round-trip partition broadcast of 1/denom).
  Phase 3: S-pass of pair B (reuses the single es buffer).
  Phase 4: PV-pass of pair B with the output projection interleaved.

Self-contained: hardcodes all shapes; inputs are the full unsharded tensors.
"""
import sys

if "/opt/trn_rl_repo" not in sys.path:
    sys.path.insert(0, "/opt/trn_rl_repo")

import numpy as np
import ml_dtypes

T = 2048
D = 1024
HD = 64
NH = 4          # q heads per core
TT = 16         # t-tiles of 128
EPS = 1e-6

# exp(x) ~= p(x/32)^32, deg-4 poly (rel err ~3e-3 over x in [-24, 14])
_EC = [0.99997553, 1.00007219, 0.50119073, 0.16710094, 0.03494652]
EXP_A1 = _EC[1] / _EC[0] / 32.0
EXP_A2 = _EC[2] / _EC[0] / 32.0 ** 2
EXP_A3 = _EC[3] / _EC[0] / 32.0 ** 3
EXP_A4 = _EC[4] / _EC[0] / 32.0 ** 4
EXP_CLAMP = 1.5625
# rsqrt seed y0 = r*(a + b*ms), minimax over ms in [0.15, 1.4]
RSQ_A = 0.3134030842612949
RSQ_B = 0.6838936760047243

# k-tiles whose exp is routed to the DVE, per pair (pair B runs while the
# DVE is otherwise idle, so it takes a larger share)
DVE_KTS = {0: (2, 7, 12), 1: (1, 3, 7, 12)}
import os
# Custom DVE ops crash NRT on this image (dispatch rows beyond the shipped
# ucode table); keep them disabled unless explicitly re-enabled.
USE_DVE_EXP = os.environ.get("K_DVE_EXP", "") == "1"
USE_DVE_RSQRT = os.environ.get("K_DVE_RSQRT", "") == "1"
# rstd via exp(-0.5*ln(ms)) keeps ACT on the natural_log_exp_and_others
# table set (square/ln/exp/copy all in one set -> no table reloads)
RSTD_LNEXP = os.environ.get("K_NO_LNEXP", "") != "1"
if not USE_DVE_EXP:
    DVE_KTS = {0: (), 1: ()}

_CACHE = {}


def _register_dve_ops():
    from concourse import dve_ops as DO
    from concourse.dve_spec import (Spec, Src0, Src1, C0, C1, C2, One, Zero,
                                    lower, sq, maxx, minn, _has_src1)
    from concourse.dve_uop import DveOpSpec

    if "ANT_EXP32P" in DO._SUB_OPCODE_FOR_NAME:
        return

    # OP: p = ((((c4*x + c3)*x + c2)*x + c1)*x + 1 ; c4=Src1, c3=imm2,
    # c2=s1, c1=s0
    b_poly = ((((Src1 * Src0) + C2) * Src0 + C1) * Src0 + C0) * Src0 + One

    def r_poly(in0, in1, s0, s1, imm2):
        x = np.asarray(in0, np.float32)
        c4 = np.asarray(in1, np.float32)
        c4 = c4.reshape(c4.shape[:1] + (1,) * (x.ndim - 1))
        return ((((c4 * x) + np.float32(imm2)) * x + np.float32(s1)) * x
                + np.float32(s0)) * x + np.float32(1.0)

    # OP: clamp(x, 0, s0)^32
    b_sq32 = sq(sq(sq(sq(sq(minn(maxx(Src0, Zero), C0))))))

    def r_sq32(in0, in1, s0, s1, imm2):
        x = np.clip(np.asarray(in0, np.float32), 0.0, s0)
        for _ in range(5):
            x = (x * x).astype(np.float32)
        return x

    # OP: rsqrt seed y0 = Src1*(C0 + C1*Src0)
    b_rsqs = Src1 * (C0 + C1 * Src0)

    def r_rsqs(in0, in1, s0, s1, imm2):
        return np.asarray(in1, np.float32) * (
            np.float32(s0) + np.float32(s1) * np.asarray(in0, np.float32))

    # OP: one Newton step 0.5*y*(3 - x*y^2); in0=x, in1=y, s0=3, s1=0.5
    b_rsqn = (Src1 * (C0 - Src0 * sq(Src1))) * C1

    def r_rsqn(in0, in1, s0, s1, imm2):
        x = np.asarray(in0, np.float32)
        y = np.asarray(in1, np.float32)
        return (y * (np.float32(s0) - x * y * y)) * np.float32(s1)

    specs = [("ANT_EXP32P", Spec(body=b_poly, reference=r_poly)),
             ("ANT_SQ32", Spec(body=b_sq32, reference=r_sq32)),
             ("ANT_RSQS", Spec(body=b_rsqs, reference=r_rsqs)),
             ("ANT_RSQN", Spec(body=b_rsqn, reference=r_rsqn))]
    for nm, sp in specs:
        row = DO._CUSTOM_DVE_ROW_BASE + len(DO.OPS)
        assert row < 0x20
        shas = {}
        for ver in ("v3", "v4"):
            tmp = DveOpSpec(name=nm, opcode=row, uops=lower(sp, ver=ver),
                            rd1_en=_has_src1(sp))
            shas[ver] = tmp.sha(ver)
        DO.OPS.append(DO.DveOp(nm, sp, subdim=False, uops_sha=shas))
        DO._SUB_OPCODE_FOR_NAME[nm] = row
        DO.CUSTOM_DVE_SPECS[nm] = sp


def _sgroups(kt):
    """Exp groups for k-tile kt: [(c0, c1)] global col spans, <=1024 wide."""
    out = []
    c = 128 * kt
    while c < T:
        out.append((c, min(c + 1024, T)))
        c += 1024
    return out


def _es_offs():
    offs, tot = {}, 0
    for kt in range(TT):
        offs[kt] = tot
        tot += T - 128 * kt
    return offs, tot


ES_OFF, ES_TOT = _es_offs()


def _build_nc(variant="full"):
    import concourse.bass as bass
    import concourse.tile as tile
    from concourse import bacc, mybir
    from concourse.masks import make_identity

    _register_dve_ops()
    from concourse import dve_ops as DO
    OP_EXP = next(o for o in DO.OPS if o.name == "ANT_EXP32P")
    OP_SQ32 = next(o for o in DO.OPS if o.name == "ANT_SQ32")
    OP_RSQS = next(o for o in DO.OPS if o.name == "ANT_RSQS")
    OP_RSQN = next(o for o in DO.OPS if o.name == "ANT_RSQN")

    F32 = mybir.dt.float32
    BF16 = mybir.dt.bfloat16
    AF = mybir.ActivationFunctionType
    AX = mybir.AxisListType

    nc = bacc.Bacc("TRN2", target_bir_lowering=False, debug=False,
                   num_devices=8)

    xT_d = nc.dram_tensor("xT", [D, T], BF16, kind="ExternalInput")
    wqkvT_d = nc.dram_tensor("wqkvT", [128, 8 * 384], BF16,
                             kind="ExternalInput")
    woT_d = nc.dram_tensor("woT", [128, 2 * 1024], BF16,
                           kind="ExternalInput")
    cc_d = nc.dram_tensor("cc", [T, 320], BF16, kind="ExternalInput")
    ss_d = nc.dram_tensor("ss", [T, 320], BF16, kind="ExternalInput")
    out_d = nc.dram_tensor("out", [T, D], BF16, kind="ExternalOutput")

    with tile.TileContext(nc) as tc:
        with tc.tile_pool(name="singles", bufs=1) as singles, \
             tc.tile_pool(name="cnorm", bufs=3) as cnorm, \
             tc.tile_pool(name="tmpx", bufs=2) as tmpx, \
             tc.tile_pool(name="dscr", bufs=4, space="DRAM") as dscr:
            # --- persistent SBUF tensors ---
            ident = singles.tile([128, 128], BF16)
            make_identity(nc, ident)
            M = singles.tile([128, 128], BF16)  # wedge: 1.0 if p <= c
            nc.gpsimd.memset(M, 1.0)
            nc.gpsimd.affine_select(
                out=M, in_=M, compare_op=mybir.AluOpType.is_ge, fill=0.0,
                base=0, channel_multiplier=-1, pattern=[[1, 128]])

            wqkv = singles.tile([128, 8, 384], BF16)
            nc.sync.dma_start(
                out=wqkv, in_=wqkvT_d[:, :].rearrange("p (c n) -> p c n", c=8))
            wo = singles.tile([128, 2, 1024], BF16)
            nc.sync.dma_start(
                out=wo, in_=woT_d[:, :].rearrange("p (c n) -> p c n", c=2))

            xT = []
            for c in range(8):
                xt = singles.tile([128, T], BF16, tag=f"xT{c}")
                xT.append(xt)
            for quarter in range(4):
                hs = slice(512 * quarter, 512 * (quarter + 1))
                for c in range(8):
                    nc.sync.dma_start(
                        out=xT[c][:, hs],
                        in_=xT_d[128 * c:128 * (c + 1), hs])

            # qkT[:, p, :]: q heads pair p (h_even at partitions 0-63,
            # h_odd at 64-127); qkT[:, 2, :]: kv head dims duplicated 2x
            qkT = singles.tile([128, 3, T], BF16)
            v1 = singles.tile([128, TT, 128], BF16)  # V + ones col + zeros
            nc.vector.memset(v1[:, :, 64:65], 1.0)
            nc.vector.memset(v1[:, :, 65:128], 0.0)
            attnT = singles.tile([128, 2, T], BF16)
            es = singles.tile([128, 2, ES_TOT], BF16)
            c4_t = singles.tile([128, 1], F32)
            nc.vector.memset(c4_t, EXP_A4)

            # ---------- issue helpers ----------
            def issue_sgroup(pair, kt, g0, g1, sc_pool, act_dve):
                """Scores + exp for cols [g0, g1) of k-tile kt, both heads of
                `pair` concurrently via row groups. act_dve: 'act' or 'dve'."""
                sc = sc_pool.tile([128, 2, 1024], F32, tag="sc", name="sc")
                ks = slice(128 * kt, 128 * (kt + 1))
                mm0 = g0
                while mm0 < g1:
                    mm1 = min(mm0 + 512, g1)
                    for h in range(2):
                        hp = 64 * h
                        nc.tensor.matmul(
                            sc[:, h, mm0 - g0:mm1 - g0],
                            qkT[hp:hp + 64, 2, ks],
                            qkT[hp:hp + 64, pair, mm0:mm1],
                            start=True, stop=True)
                    mm0 = mm1
                eo = ES_OFF[kt] + (g0 - 128 * kt)
                G = g1 - g0
                if act_dve == "act":
                    for h in range(2):
                        nc.scalar.activation(out=es[:, h, eo:eo + G],
                                             in_=sc[:, h, 0:G], func=AF.Exp)
                else:
                    tx = tmpx.tile([128, 2, 1024], F32, tag="tx", name="tx")
                    for h in range(2):
                        nc.vector._custom_dve(
                            OP_EXP, out=tx[:, h, 0:G], in0=sc[:, h, 0:G],
                            in1=c4_t[:, :], s0=EXP_A1, s1=EXP_A2, imm2=EXP_A3)
                        nc.vector._custom_dve(
                            OP_SQ32, out=es[:, h, eo:eo + G],
                            in0=tx[:, h, 0:G], s0=EXP_CLAMP)
                if g0 == 128 * kt:  # wedge lives in first 128 cols
                    for h in range(2):
                        nc.vector.tensor_mul(es[:, h, eo:eo + 128],
                                             es[:, h, eo:eo + 128], M)

            def s_units(pair, sc_pool):
                """Closures issuing the S-pass of `pair`, kt 15..0."""
                units = []
                for kt in range(TT - 1, -1, -1):
                    for g0, g1 in _sgroups(kt):
                        route = "dve" if kt in DVE_KTS[pair] else "act"
                        units.append(
                            lambda p=pair, k=kt, a=g0, b=g1, r=route:
                            issue_sgroup(p, k, a, b, sc_pool, r))
                return units

            def s_units_for_kt(pair, kt, sc_pool):
                units = []
                for g0, g1 in _sgroups(kt):
                    route = "dve" if kt in DVE_KTS[pair] else "act"
                    units.append(
                        lambda p=pair, k=kt, a=g0, b=g1, r=route:
                        issue_sgroup(p, k, a, b, sc_pool, r))
                return units

            def pv_units(pair, pv_pool, pv_tiles, norm_cb):
                """Yield closures for the PV-pass of `pair`: chunk-j psum
                accumulation over kt, then normalization."""
                for j in range(4):
                    for kt in range(4 * j + 4):
                        def mk(jj=j, kk=kt, pp=pair):
                            if kk == 0:
                                pv_tiles[jj] = pv_pool.tile(
                                    [128, 1024], F32, tag="pv",
                                    name=f"pv{pp}_{jj}")
                            pv = pv_tiles[jj]
                            w0c = max(0, 128 * kk - 512 * jj)
                            a = ES_OFF[kk] + 512 * jj + w0c - 128 * kk
                            n = 512 - w0c
                            last = kk == 4 * jj + 3
                            for h in range(2):
                                nc.tensor.matmul(
                                    pv[0:128, 512 * h + w0c:512 * (h + 1)],
                                    v1[:, kk, :],
                                    es[:, h, a:a + n],
                                    start=(kk == 0), stop=last,
                                    skip_group_check=not (kk == 0 or last))
                        yield mk
                    yield lambda jj=j, pp=pair: norm_cb(pp, jj, pv_tiles[jj])

            def issue_norm(pair, j, pv):
                """1/denominator via DRAM-round-trip broadcast, then scale."""
                dcp = cnorm.tile([1, 1024], F32, tag="dcp", name="dcp")
                nc.vector.tensor_copy(dcp, pv[64:65, :])
                rc = cnorm.tile([1, 1024], F32, tag="rc", name="rc")
                nc.vector.reciprocal_approx_fast(out=rc, in_=dcp)
                scr = dscr.tile([1, 1024], F32, tag="scr", name="scr")
                nc.sync.dma_start(out=scr, in_=rc)
                rcb = cnorm.tile([64, 1024], F32, tag="rcb", name="rcb")
                src = bass.AP(tensor=scr.tensor, offset=scr.offset,
                              ap=[[0, 64], [1, 1024]])
                nc.sync.dma_start(out=rcb, in_=src)
                cs_ = slice(512 * j, 512 * (j + 1))
                nc.vector.tensor_mul(attnT[0:64, pair, cs_],
                                     pv[0:64, 0:512], rcb[:, 0:512])
                nc.vector.tensor_mul(attnT[64:128, pair, cs_],
                                     pv[0:64, 512:1024], rcb[:, 512:1024])

            def issue_d(i, po_pool, dwork):
                dts = slice(128 * i, 128 * (i + 1))
                ob = dwork.tile([128, 1024], BF16, tag="ob", name="ob")
                for nh in range(2):
                    po = po_pool.tile([128, 512], F32, tag="po",
                                      name=f"po{nh}")
                    for c in range(2):
                        nc.tensor.matmul(
                            po, attnT[:, c, dts],
                            wo[:, c, 512 * nh:512 * (nh + 1)],
                            start=(c == 0), stop=(c == 1))
                    if nh == 0:
                        nc.scalar.copy(ob[:, 0:512], po)
                    else:
                        nc.vector.tensor_copy(ob[:, 512:1024], po)
                nc.sync.dma_start(out=out_d[dts, :], in_=ob)

            # ================= PHASE 1: B (tiles 15..0) + S(pair A) ========
            with tc.tile_pool(name="ps_proj", bufs=2, space="PSUM") as ps_proj, \
                 tc.tile_pool(name="ps_tr", bufs=2, space="PSUM") as ps_tr, \
                 tc.tile_pool(name="ps_sc1", bufs=1, space="PSUM") as ps_sc1, \
                 tc.tile_pool(name="bwork", bufs=2) as bwork, \
                 tc.tile_pool(name="bsmall", bufs=2) as bsmall:
                pqs = {}
                ropes = {}
                for it in range(TT + 4):
                    # ---- stage 0: proj for tile 15-it ----
                    if it < TT:
                        i = TT - 1 - it
                        ts = slice(128 * i, 128 * (i + 1))
                        cs = bwork.tile([128, 320], BF16, tag="cs")
                        nc.sync.dma_start(out=cs, in_=cc_d[ts, :])
                        sn = bwork.tile([128, 320], BF16, tag="sn")
                        nc.sync.dma_start(out=sn, in_=ss_d[ts, :])
                        pq = ps_proj.tile([128, 384], F32, tag="pq")
                        for c in range(8):
                            nc.tensor.matmul(
                                pq, xT[c][:, ts], wqkv[:, c, :],
                                start=(c == 0), stop=(c == 7))
                        pqs[i] = (pq, cs, sn)
                    # ---- stage 1 (lag 1): stats + rope for tile 15-(it-1) --
                    if 1 <= it <= TT:
                        i = TT - it
                        pq, cs, sn = pqs.pop(i)
                        sq_t = bwork.tile([128, 320], F32, tag="sq")
                        nc.scalar.activation(out=sq_t, in_=pq[:, 0:320],
                                             func=AF.Square)
                        ssum = bsmall.tile([128, 5], F32, tag="ssum")
                        nc.vector.reduce_sum(
                            out=ssum,
                            in_=sq_t.rearrange("p (h d) -> p h d", h=5),
                            axis=AX.X)
                        msx = bsmall.tile([128, 5], F32, tag="msx")
                        nc.vector.tensor_scalar(
                            out=msx, in0=ssum, scalar1=1.0 / HD, scalar2=EPS,
                            op0=mybir.AluOpType.mult, op1=mybir.AluOpType.add)
                        if USE_DVE_RSQRT:
                            rr = bsmall.tile([128, 5], F32, tag="rr")
                            nc.vector.reciprocal_approx_fast(out=rr, in_=msx)
                            y0 = bsmall.tile([128, 5], F32, tag="y0")
                            nc.vector._custom_dve(OP_RSQS, out=y0, in0=msx,
                                                  in1=rr, s0=RSQ_A, s1=RSQ_B)
                            y1 = bsmall.tile([128, 5], F32, tag="y1")
                            nc.vector._custom_dve(OP_RSQN, out=y1, in0=msx,
                                                  in1=y0, s0=3.0, s1=0.5)
                            y2 = bsmall.tile([128, 5], F32, tag="y2")
                            nc.vector._custom_dve(OP_RSQN, out=y2, in0=msx,
                                                  in1=y1, s0=3.0, s1=0.5)
                            rstd = bsmall.tile([128, 5], F32, tag="rstd")
                            nc.vector._custom_dve(OP_RSQN, out=rstd, in0=msx,
                                                  in1=y2, s0=3.0, s1=0.5)
                        elif RSTD_LNEXP:
                            lg = bsmall.tile([128, 5], F32, tag="lg")
                            nc.scalar.activation(out=lg, in_=msx, func=AF.Ln)
                            rstd = bsmall.tile([128, 5], F32, tag="rstd")
                            nc.scalar.activation(out=rstd, in_=lg,
                                                 func=AF.Exp, scale=-0.5)
                        else:
                            stdv = bsmall.tile([128, 5], F32, tag="stdv")
                            nc.scalar.activation(out=stdv, in_=msx,
                                                 func=AF.Sqrt)
                            rstd = bsmall.tile([128, 5], F32, tag="rstd")
                            nc.vector.reciprocal_approx_fast(out=rstd,
                                                             in_=stdv)
                        rstd_b = bass.AP(
                            tensor=rstd.tensor, offset=rstd.offset,
                            ap=[rstd.ap[0], [1, 5], [0, 64]])
                        qs = bwork.tile([128, 320], BF16, tag="qs")
                        nc.vector.tensor_mul(qs, pq[:, 0:320], rstd_b)
                        nc.scalar.copy(v1[:, i, 0:64], pq[:, 320:384])
                        xc = bwork.tile([128, 320], BF16, tag="xc")
                        nc.vector.tensor_mul(xc, qs, cs)
                        qs_swap = bass.AP(
                            tensor=qs.tensor, offset=qs.offset + 32,
                            ap=[qs.ap[0], [64, 5], [-32, 2], [1, 32]])
                        xs = bwork.tile([128, 5, 64], BF16, tag="xs")
                        nc.vector.tensor_mul(xs, qs_swap, sn.rearrange(
                            "p (h d) -> p h d", h=5))
                        rope = bwork.tile([128, 320], BF16, tag="rope")
                        nc.vector.tensor_add(
                            rope, xc, xs.rearrange("p h d -> p (h d)"))
                        ropes[i] = rope
                    # ---- stage 2 (lag 2): transpose + copyout ----
                    if 2 <= it <= TT + 1:
                        i = TT + 1 - it
                        ts = slice(128 * i, 128 * (i + 1))
                        rope = ropes.pop(i)
                        tp = ps_tr.tile([128, 384], BF16, tag="tp")
                        nc.tensor.transpose(tp[:, 0:128], rope[:, 0:128],
                                            ident)
                        nc.tensor.transpose(tp[:, 128:256], rope[:, 128:256],
                                            ident)
                        ktr = bwork.tile([128, 128], BF16, tag="ktr")
                        nc.vector.tensor_copy(ktr[:, 0:64], rope[:, 256:320])
                        nc.gpsimd.tensor_copy(ktr[:, 64:128],
                                              rope[:, 256:320])
                        nc.tensor.transpose(tp[:, 256:384], ktr, ident)
                        nc.vector.tensor_copy(qkT[:, :, ts], tp.rearrange(
                            "p (c n) -> p c n", c=3))
                    # ---- stage 3 (lag to copyout of tile kt): S(A) for
                    # kt = TT+3-it; tile kt's qkT copy was issued at
                    # iteration it-2, tiles > kt earlier. ----
                    kt_s = TT + 3 - it
                    if 0 <= kt_s < TT:
                        for u in s_units_for_kt(0, kt_s, ps_sc1):
                            u()

            if variant == "proj":
                nc.gpsimd.dma_start(out=out_d[0:128, 0:D],
                                    in_=qkT[:, 0, 0:D])

            # ============ PHASE 2: PV(pair A) ==============================
            # (es is a single shared buffer: S(B) would overwrite it, so the
            # B-pair S-pass waits until PV(A) has consumed it.)
            with tc.tile_pool(name="ps_pv", bufs=2, space="PSUM") as ps_pv:
                pv_tiles = {}
                for u in pv_units(0, ps_pv, pv_tiles, issue_norm):
                    u()

                # ============ PHASE 3: S(pair B) ===========================
                with tc.tile_pool(name="ps_sc2", bufs=1,
                                  space="PSUM") as ps_sc2:
                    for u in s_units(1, ps_sc2):
                        u()

                if variant == "attn":
                    nc.gpsimd.dma_start(out=out_d[0:128, 0:D],
                                        in_=attnT[:, 0, 0:D])

                # ============ PHASE 4: PV(pair B) + D ======================
                # D tiles for chunk j issue after norm(B, j); interleaved
                # with chunk j+1's PV matmuls so the PE never waits on the
                # denominator DMA round-trip.
                with tc.tile_pool(name="ps_po", bufs=2,
                                  space="PSUM") as ps_po, \
                     tc.tile_pool(name="dwork", bufs=2) as dwork:
                    pv_tilesB = {}
                    pB = list(pv_units(1, ps_pv, pv_tilesB, issue_norm))
                    # pB layout: [chunk0 MMs..., norm0, chunk1 MMs..., norm1,
                    #  ...]; chunk j has 4j+4 MM units + 1 norm.
                    idx = 0
                    pending_d = []
                    for j in range(4):
                        nmm = 4 * j + 4
                        for u in pB[idx:idx + nmm]:
                            u()
                            if pending_d:
                                issue_d(pending_d.pop(0), ps_po, dwork)
                        pB[idx + nmm]()  # norm(B, j)
                        idx += nmm + 1
                        pending_d += [4 * j, 4 * j + 1, 4 * j + 2, 4 * j + 3]
                    for i in pending_d:
                        issue_d(i, ps_po, dwork)

            if variant == "dump":
                nc.gpsimd.dma_start(out=out_d[0:128, 0:D],
                                    in_=qkT[:, 0, 0:D])
                nc.gpsimd.dma_start(out=out_d[128:256, 0:D],
                                    in_=qkT[:, 2, 0:D])
                nc.gpsimd.dma_start(out=out_d[256:384, 0:D],
                                    in_=attnT[:, 0, 0:D])
    nc.compile()
    return nc


def _host_tables(cos, sin, qn_w, kn_w):
    scale = HD ** -0.5
    cch = np.concatenate([cos, cos], 1).astype(np.float32)         # (T, 64)
    ssh = np.concatenate([-sin, sin], 1).astype(np.float32)
    qn4 = np.tile(qn_w, 4).astype(np.float32)
    swq4 = np.tile(np.concatenate([qn_w[32:], qn_w[:32]]), 4).astype(np.float32)
    swk = np.concatenate([kn_w[32:], kn_w[:32]]).astype(np.float32)
    cc = np.concatenate(
        [np.tile(cch, (1, NH)) * qn4[None] * scale, cch * kn_w[None]], 1)
    ss = np.concatenate(
        [np.tile(ssh, (1, NH)) * swq4[None] * scale, ssh * swk[None]], 1)
    return (np.ascontiguousarray(cc).astype(ml_dtypes.bfloat16),
            np.ascontiguousarray(ss).astype(ml_dtypes.bfloat16))


def make_in_maps(x, cos, sin, wq, wk, wv, wo, qn_w, kn_w):
    cc, ss = _host_tables(cos, sin, qn_w, kn_w)
    in_maps = []
    for core in range(8):
        b, g = divmod(core, 4)
        wqkvT = np.concatenate(
            [wq[256 * g:256 * (g + 1)],
             wk[64 * g:64 * (g + 1)],
             wv[64 * g:64 * (g + 1)]], 0).T.astype(ml_dtypes.bfloat16)
        wqkvT = np.ascontiguousarray(
            wqkvT.reshape(8, 128, 384).transpose(1, 0, 2).reshape(128, -1))
        woT = wo[:, 256 * g:256 * (g + 1)].T.astype(ml_dtypes.bfloat16)
        woT = np.ascontiguousarray(
            woT.reshape(2, 128, 1024).transpose(1, 0, 2).reshape(128, -1))
        xT = np.ascontiguousarray(np.asarray(x)[b].T.astype(ml_dtypes.bfloat16))
        in_maps.append({"xT": xT, "wqkvT": wqkvT, "woT": woT,
                        "cc": cc, "ss": ss})
    return in_maps


def kernel(x, cos, sin, wq, wk, wv, wo, qn_w, kn_w):
    from concourse.bass_utils import run_bass_kernel_spmd

    if "nc" not in _CACHE:
        _CACHE["nc"] = _build_nc()
    nc = _CACHE["nc"]
    in_maps = make_in_maps(np.asarray(x), np.asarray(cos), np.asarray(sin),
                           np.asarray(wq), np.asarray(wk), np.asarray(wv),
                           np.asarray(wo), np.asarray(qn_w), np.asarray(kn_w))
    res = run_bass_kernel_spmd(nc, in_maps, core_ids=list(range(8)))
    out = np.zeros((2, T, D), np.float32)
    for core in range(8):
        b = core // 4
        out[b] += res.results[core]["out"].astype(np.float32)
    return out


# revision 19
# speedup vs baseline: 1.2771x; 1.2771x over previous
"""GQA attention (16 q-heads / 4 kv-heads, head_dim 64, T=2048, D=1024) on 8
Trainium2 NeuronCores.

Sharding: 8 shards = batch(2) x kv-group(4). Each core handles one batch
element and one whole GQA group (4 query heads + their shared kv head), and
computes a partial output projection; the host sums the 4 group-partials per
batch element.

Pipeline per core:
  B: qkv proj (psum-accumulated over 8 c-chunks), RMSNorm via ACT-square +
     reciprocal_approx_fast, RoPE in bf16 with a 0-stride rstd broadcast AP,
     PE transposes into a fused [128,384] psum tile; 3-stage software
     pipeline (proj | stats+rope | transpose+copyout) so no engine FIFO
     waits on the previous tile's cross-engine chain.
  C: per (head, k-tile): exact-causal score matmuls into [128,1024] psum
     windows, one EXP per window on ACT, wedge mask on DVE; PV accumulates
     per-512-column chunks with a lag-2 software pipeline; softmax denom
     normalized via reciprocal_approx_fast + DRAM-round-trip partition
     broadcast (gpsimd partition_broadcast gives wrong data on HW).
  D: output projection, bf16 store, host sums partials in fp32.

Self-contained: hardcodes all shapes; inputs are the full unsharded tensors.
"""
import sys

if "/opt/trn_rl_repo" not in sys.path:
    sys.path.insert(0, "/opt/trn_rl_repo")

import numpy as np
import ml_dtypes

T = 2048
D = 1024
HD = 64
NH = 4          # q heads per core
TT = 16         # t-tiles of 128
EPS = 1e-6

_CACHE = {}


def _build_nc(variant="full"):
    import concourse.bass as bass
    import concourse.tile as tile
    from concourse import bacc, mybir
    from concourse.masks import make_identity

    F32 = mybir.dt.float32
    BF16 = mybir.dt.bfloat16
    AF = mybir.ActivationFunctionType
    AX = mybir.AxisListType

    nc = bacc.Bacc("TRN2", target_bir_lowering=False, debug=False,
                   num_devices=8)

    xT_d = nc.dram_tensor("xT", [D, T], BF16, kind="ExternalInput")
    # weights pre-tiled on host to [128, c, n] so the DMA is contiguous
    wqkvT_d = nc.dram_tensor("wqkvT", [128, 8 * 384], BF16,
                             kind="ExternalInput")
    woT_d = nc.dram_tensor("woT", [128, 2 * 1024], BF16,
                           kind="ExternalInput")
    cc_d = nc.dram_tensor("cc", [T, 320], BF16, kind="ExternalInput")
    ss_d = nc.dram_tensor("ss", [T, 320], BF16, kind="ExternalInput")
    out_d = nc.dram_tensor("out", [T, D], BF16, kind="ExternalOutput")

    with tile.TileContext(nc) as tc:
        with tc.tile_pool(name="singles", bufs=1) as singles:
            # --- persistent SBUF tensors ---
            ident = singles.tile([128, 128], BF16)
            make_identity(nc, ident)
            # wedge mask M[p, c] = 1.0 if p <= c else 0.0
            M = singles.tile([128, 128], BF16)
            nc.gpsimd.memset(M, 1.0)
            nc.gpsimd.affine_select(
                out=M, in_=M, compare_op=mybir.AluOpType.is_ge, fill=0.0,
                base=0, channel_multiplier=-1, pattern=[[1, 128]])

            wqkv = singles.tile([128, 8, 384], BF16)
            nc.sync.dma_start(
                out=wqkv, in_=wqkvT_d[:, :].rearrange("p (c n) -> p c n", c=8))
            wo = singles.tile([128, 2, 1024], BF16)
            nc.sync.dma_start(
                out=wo, in_=woT_d[:, :].rearrange("p (c n) -> p c n", c=2))

            xT = []
            for c in range(8):
                xt = singles.tile([128, T], BF16, tag=f"xT{c}")
                xT.append(xt)
            for quarter in range(4):
                hs = slice(512 * quarter, 512 * (quarter + 1))
                for c in range(8):
                    nc.sync.dma_start(
                        out=xT[c][:, hs],
                        in_=xT_d[128 * c:128 * (c + 1), hs])

            # qkT[:, 0, :] = heads {0,2} dims, qkT[:, 1, :] = heads {1,3},
            # qkT[:, 2, :] = kv head dims duplicated 2x
            qkT = singles.tile([128, 3, T], BF16)
            v1 = singles.tile([128, TT, 65], BF16)     # V tiles + ones col
            nc.vector.memset(v1[:, :, 64:65], 1.0)
            attnT = singles.tile([128, 2, T], BF16)    # normalized attn out^T
            eps_t = singles.tile([128, 1], F32)
            nc.vector.memset(eps_t, EPS)

            # ---------------- Phase B: proj + rmsnorm + rope + transpose -----
            with tc.tile_pool(name="ps_proj", bufs=4, space="PSUM") as ps_proj, \
                 tc.tile_pool(name="ps_tr", bufs=3, space="PSUM") as ps_tr, \
                 tc.tile_pool(name="bwork", bufs=6) as bwork, \
                 tc.tile_pool(name="bsmall", bufs=6) as bsmall:
                # 3-stage software pipeline: proj(i) | stats+rope(i-2) |
                # transpose+copyout(i-3).  Keeps each engine FIFO free of
                # waits on the previous tile's cross-engine chain.
                pqs = {}
                ropes = {}
                for it in range(TT + 3):
                    if it < TT:
                        i = it
                        ts = slice(128 * i, 128 * (i + 1))
                        cs = bwork.tile([128, 320], BF16, tag="cs")
                        nc.sync.dma_start(out=cs, in_=cc_d[ts, :])
                        sn = bwork.tile([128, 320], BF16, tag="sn")
                        nc.sync.dma_start(out=sn, in_=ss_d[ts, :])
                        pq = ps_proj.tile([128, 384], F32, tag="pq")
                        for c in range(8):
                            nc.tensor.matmul(
                                pq, xT[c][:, ts], wqkv[:, c, :],
                                start=(c == 0), stop=(c == 7))
                        pqs[i] = (pq, cs, sn)
                    if 2 <= it and it - 2 < TT:
                        i = it - 2
                        pq, cs, sn = pqs.pop(i)
                        sq = bwork.tile([128, 320], F32, tag="sq")
                        nc.scalar.activation(out=sq, in_=pq[:, 0:320],
                                             func=AF.Square)
                        ssum = bsmall.tile([128, 5], F32, tag="ssum")
                        nc.vector.reduce_sum(
                            out=ssum,
                            in_=sq.rearrange("p (h d) -> p h d", h=5),
                            axis=AX.X)
                        stdv = bsmall.tile([128, 5], F32, tag="stdv")
                        nc.scalar.activation(
                            out=stdv, in_=ssum, func=AF.Sqrt,
                            bias=eps_t[:, :], scale=1.0 / HD)
                        rstd = bsmall.tile([128, 5], F32, tag="rstd")
                        nc.vector.reciprocal_approx_fast(out=rstd, in_=stdv)
                        rstd_b = bass.AP(
                            tensor=rstd.tensor, offset=rstd.offset,
                            ap=[rstd.ap[0], [1, 5], [0, 64]])
                        qs = bwork.tile([128, 320], BF16, tag="qs")
                        nc.vector.tensor_mul(qs, pq[:, 0:320], rstd_b)
                        nc.vector.tensor_copy(v1[:, i, 0:64], pq[:, 320:384])
                        xc = bwork.tile([128, 320], BF16, tag="xc")
                        nc.vector.tensor_mul(xc, qs, cs)
                        qs_swap = bass.AP(
                            tensor=qs.tensor, offset=qs.offset + 32,
                            ap=[qs.ap[0], [64, 5], [-32, 2], [1, 32]])
                        xs = bwork.tile([128, 5, 64], BF16, tag="xs")
                        nc.vector.tensor_mul(xs, qs_swap, sn.rearrange(
                            "p (h d) -> p h d", h=5))
                        rope = bwork.tile([128, 320], BF16, tag="rope")
                        nc.vector.tensor_add(
                            rope, xc, xs.rearrange("p h d -> p (h d)"))
                        ktr = bwork.tile([128, 128], BF16, tag="ktr")
                        nc.gpsimd.tensor_copy(ktr[:, 0:64], rope[:, 256:320])
                        nc.gpsimd.tensor_copy(ktr[:, 64:128],
                                              rope[:, 256:320])
                        ropes[i] = (rope, ktr)
                    if it < 3:
                        continue
                    i = it - 3
                    ts = slice(128 * i, 128 * (i + 1))
                    rope, ktr = ropes.pop(i)
                    tp = ps_tr.tile([128, 384], BF16, tag="tp")
                    nc.tensor.transpose(tp[:, 0:128], rope[:, 0:128], ident)
                    nc.tensor.transpose(tp[:, 128:256], rope[:, 128:256],
                                        ident)
                    nc.tensor.transpose(tp[:, 256:384], ktr, ident)
                    nc.scalar.copy(qkT[:, :, ts], tp.rearrange(
                        "p (c n) -> p c n", c=3))

            if variant == "proj":
                nc.gpsimd.dma_start(out=out_d[0:128, 0:D],
                                    in_=qkT[:, 0, 0:D])
            # ---------------- Phase C: attention ----------------------------
            # per (h, kt): score windows of <=1024 cols covering
            # [512*(kt>>2), 2048); exact-causal col starts at 128*kt.
            # pv[j] accumulates chunk j = cols [512j, 512j+512) over kt<=4j+3.
            with tc.tile_pool(name="ps_sc", bufs=2, space="PSUM") as ps_sc, \
                 tc.tile_pool(name="ps_pv", bufs=1, space="PSUM") as ps_pv, \
                 tc.tile_pool(name="es_pool", bufs=8) as es_pool, \
                 tc.tile_pool(name="cnorm", bufs=4) as cnorm, \
                 tc.tile_pool(name="dwork", bufs=3) as dwork, \
                 tc.tile_pool(name="dscr", bufs=4, space="DRAM") as dscr:

                def issue_d(i):
                    # output projection for t-tile i; po reuses the pv0/pv1
                    # psum slots (safe once the h3 norms for the needed
                    # chunks are issued)
                    dts = slice(128 * i, 128 * (i + 1))
                    ob = dwork.tile([128, 1024], BF16, tag="ob", name="ob")
                    for nh in range(2):
                        po = ps_pv.tile([128, 512], F32, tag=f"pv{nh}",
                                        name=f"po{nh}")
                        for c in range(2):
                            nc.tensor.matmul(
                                po,
                                attnT[:, c, dts],
                                wo[:, c, 512 * nh:512 * (nh + 1)],
                                start=(c == 0), stop=(c == 1))
                        if nh == 0:
                            nc.scalar.copy(ob[:, 0:512], po)
                        else:
                            nc.vector.tensor_copy(ob[:, 512:1024], po)
                    nc.sync.dma_start(out=out_d[dts, :], in_=ob)

                # D tiles interleaved into head 3's tail: tile i (chunk
                # j=i>>2) needs h3 norm(j) issued, and po's pv1-slot reuse
                # needs norm(1) issued (kti=9) -> start at kti=10
                d_sched = {10: [0, 1], 11: [2, 3], 12: [4, 5], 13: [6, 7],
                           14: [8, 9], 15: [10, 11]}
                for h in range(NH):
                    hp = 64 * (h % 2)
                    hc = h // 2
                    pv_t = {}
                    es_ref = {}   # (kt, j) -> (es_tile, local col offset)
                    for kti in range(18):
                        # ---- scores + exp for kt = kti ----
                        if kti < 16:
                            kt = kti
                            j0 = kt >> 2
                            base = 512 * j0
                            q0 = 128 * kt          # first valid q col
                            wstarts = [base] if base + 1024 >= 2048 \
                                else [base, base + 1024]
                            for ws in wstarts:
                                we = min(ws + 1024, 2048)
                                sc = ps_sc.tile([128, 1024], F32, tag="sc")
                                es = es_pool.tile([128, 1024], BF16, tag="es")
                                lo = max(ws, q0)
                                # matmuls split at 512 boundaries (psum bank)
                                mstart = lo
                                while mstart < we:
                                    mend = min((mstart // 512 + 1) * 512, we)
                                    nc.tensor.matmul(
                                        sc[:, mstart - ws:mend - ws],
                                        qkT[hp:hp + 64, 2,
                                            128 * kt:128 * (kt + 1)],
                                        qkT[hp:hp + 64, hc, mstart:mend],
                                        start=True, stop=True)
                                    mstart = mend
                                nc.scalar.activation(
                                    out=es[:, lo - ws:we - ws],
                                    in_=sc[:, lo - ws:we - ws], func=AF.Exp)
                                if lo == q0:  # wedge tile lives here
                                    nc.vector.tensor_mul(
                                        es[:, lo - ws:lo - ws + 128],
                                        es[:, lo - ws:lo - ws + 128], M)
                                for j in range(max(j0, ws // 512),
                                               (we + 511) // 512):
                                    es_ref[(kt, j)] = (es, 512 * j - ws)
                        # ---- pv for kt = kti - 2 ----
                        if kti >= 2:
                            kp = kti - 2
                            j0p = kp >> 2
                            if kp == 0:
                                for j in range(4):
                                    pv_t[j] = ps_pv.tile(
                                        [128, 512], F32, tag=f"pv{j}",
                                        name=f"pv{j}")
                            for j in range(j0p, 4):
                                es, off = es_ref[(kp, j)]
                                w0c = 128 * (kp - 4 * j) if j == j0p else 0
                                w0c = max(w0c, 0)
                                last = (kp == 4 * j + 3)
                                nc.tensor.matmul(
                                    pv_t[j][0:65, w0c:512],
                                    v1[:, kp, :],
                                    es[:, off + w0c:off + 512],
                                    start=(kp == 0), stop=last,
                                    skip_group_check=not (kp == 0 or last))
                            # normalization for finished chunk
                            if kp % 4 == 3:
                                j = kp >> 2
                                # copy psum out early so pv_t[j] frees
                                # without waiting the DMA round-trip
                                dcp = cnorm.tile([1, 512], F32, tag="dcp")
                                nc.vector.tensor_copy(dcp, pv_t[j][64:65, :])
                                pvc = cnorm.tile([64, 512], F32, tag="pvc")
                                nc.vector.tensor_copy(pvc, pv_t[j][0:64, :])
                                rc = cnorm.tile([1, 512], F32, tag="rc")
                                nc.vector.reciprocal_approx_fast(
                                    out=rc, in_=dcp)
                                scr = dscr.tile([1, 512], F32, tag="scr",
                                                name="scr")
                                nc.sync.dma_start(out=scr, in_=rc)
                                rcb = cnorm.tile([64, 512], F32, tag="rcb")
                                src = bass.AP(
                                    tensor=scr.tensor, offset=scr.offset,
                                    ap=[[0, 64], [1, 512]])
                                nc.sync.dma_start(out=rcb, in_=src)
                                nc.vector.tensor_mul(
                                    attnT[hp:hp + 64, hc,
                                          512 * j:512 * (j + 1)],
                                    pvc, rcb)
                        if h == 3:
                            for i in d_sched.get(kti, []):
                                issue_d(i)
                for i in range(12, 16):
                    issue_d(i)

            if variant == "attn":
                nc.gpsimd.dma_start(out=out_d[0:128, 0:D],
                                    in_=attnT[:, 0, 0:D])
            if variant == "dump":
                nc.gpsimd.dma_start(out=out_d[0:128, 0:D],
                                    in_=qkT[:, 0, 0:D])
                nc.gpsimd.dma_start(out=out_d[128:256, 0:D],
                                    in_=qkT[:, 2, 0:D])
                nc.gpsimd.dma_start(out=out_d[256:384, 0:D],
                                    in_=attnT[:, 0, 0:D])
    nc.compile()
    return nc


def _host_tables(cos, sin, qn_w, kn_w):
    scale = HD ** -0.5
    cch = np.concatenate([cos, cos], 1).astype(np.float32)         # (T, 64)
    ssh = np.concatenate([-sin, sin], 1).astype(np.float32)
    qn4 = np.tile(qn_w, 4).astype(np.float32)
    swq4 = np.tile(np.concatenate([qn_w[32:], qn_w[:32]]), 4).astype(np.float32)
    swk = np.concatenate([kn_w[32:], kn_w[:32]]).astype(np.float32)
    cc = np.concatenate(
        [np.tile(cch, (1, NH)) * qn4[None] * scale, cch * kn_w[None]], 1)
    ss = np.concatenate(
        [np.tile(ssh, (1, NH)) * swq4[None] * scale, ssh * swk[None]], 1)
    return (np.ascontiguousarray(cc).astype(ml_dtypes.bfloat16),
            np.ascontiguousarray(ss).astype(ml_dtypes.bfloat16))


def make_in_maps(x, cos, sin, wq, wk, wv, wo, qn_w, kn_w):
    cc, ss = _host_tables(cos, sin, qn_w, kn_w)
    in_maps = []
    for core in range(8):
        b, g = divmod(core, 4)
        wqkvT = np.concatenate(
            [wq[256 * g:256 * (g + 1)],
             wk[64 * g:64 * (g + 1)],
             wv[64 * g:64 * (g + 1)]], 0).T.astype(ml_dtypes.bfloat16)
        wqkvT = np.ascontiguousarray(
            wqkvT.reshape(8, 128, 384).transpose(1, 0, 2).reshape(128, -1))
        woT = wo[:, 256 * g:256 * (g + 1)].T.astype(ml_dtypes.bfloat16)
        woT = np.ascontiguousarray(
            woT.reshape(2, 128, 1024).transpose(1, 0, 2).reshape(128, -1))
        xT = np.ascontiguousarray(np.asarray(x)[b].T.astype(ml_dtypes.bfloat16))
        in_maps.append({"xT": xT, "wqkvT": wqkvT, "woT": woT,
                        "cc": cc, "ss": ss})
    return in_maps


def kernel(x, cos, sin, wq, wk, wv, wo, qn_w, kn_w):
    from concourse.bass_utils import run_bass_kernel_spmd

    if "nc" not in _CACHE:
        _CACHE["nc"] = _build_nc()
    nc = _CACHE["nc"]
    in_maps = make_in_maps(np.asarray(x), np.asarray(cos), np.asarray(sin),
                           np.asarray(wq), np.asarray(wk), np.asarray(wv),
                           np.asarray(wo), np.asarray(qn_w), np.asarray(kn_w))
    res = run_bass_kernel_spmd(nc, in_maps, core_ids=list(range(8)))
    out = np.zeros((2, T, D), np.float32)
    for core in range(8):
        b = core // 4
        out[b] += res.results[core]["out"].astype(np.float32)
    return out

